# revision 13
# baseline (speedup 1.0000x reference)
"""CQVAE loss kernel for Trainium2, data-parallel over batch on 8 NeuronCores.

loss = kld(qy) + mse(gather(rzs), zs[:, :Sg]) + bias(best, best_gt)
       + bias(gather(pts), gts)
where bias(p, g) = mse(p, g) + 10 * mse(p[..., MARK, :], g[..., MARK, :]).

Design (per core, 16 of 128 batches):
- Host packs comb rows [rzs | pts(mark-cols-first) | pad->1280B] and a
  NEGATED target [zs | gts(mark-cols-first) | pad], both fp8e4m3
  (tolerance 2e-2 >> fp8 noise; rel err lands ~1e-3), halving HBM
  traffic vs bf16.
- The mapping-gather runs as 4 dma_gather ops (512 rows each, ~1.5us of
  SWDGE emission apiece) instead of 16 indirect ops (~1.8us per 128
  rows) - the decisive change, since gather emission on the gpsimd
  queue is the kernel's critical path.
- The negated target is applied two ways, chosen per chunk to balance
  engines: chunks 0-1 via indirect-CCE ops (descriptor = one 1260B tgt
  row, the only DMA-accumulate shape the ucode supports) on the
  otherwise-idle gpsimd queue; chunks 2-3 via direct loads + one DVE
  add per chunk (tgt is pre-negated so add == subtract).
- d**2 + accumulate splits between Act (zs columns [0:ACOL), Square
  with accum) and DVE (zs tail, pts, marks, kld, best); landmark
  weights use separate accumulators (marks permuted to the front of the
  pts block) or host-folded column scales (best), so no per-landmark
  reduction ops.
"""

import sys

import numpy as np

try:
    import concourse  # noqa: F401
except ImportError:  # pragma: no cover
    sys.path.insert(0, "/opt/trn_rl_repo")

import ml_dtypes

import concourse.bass as bass  # noqa: F401
import concourse.mybir as mybir
import concourse.tile as tile
from concourse import bacc, library_config
from concourse.bass_utils import run_bass_kernel_spmd

F32 = mybir.dt.float32
BF16 = mybir.dt.bfloat16
F8 = mybir.dt.float8e4
I32 = mybir.dt.int32
I16 = mybir.dt.int16
AX = mybir.AxisListType
OP = mybir.AluOpType
ACTF = mybir.ActivationFunctionType

NCORES = 8
B, S, SG, D, P, V = 128, 256, 128, 1024, 118, 64
BL = B // NCORES  # batches per core
P2 = 2 * P  # 236 point floats per row
CW = D + P2  # 1260 live row floats
CWP = 1280  # padded row width (dma_gather wants %256 bytes)
MARK = (0, 29, 88, 117)
NMARK2 = 2 * len(MARK)  # 8 mark columns
EPS = 1e-20
ALPHA = 10.0

NCH = 4  # chunks (4 slots each)
KC = BL // NCH
CHW = KC * CWP  # 5120 cols per chunk tile
NMERGE = 2  # chunks 0..NMERGE-1 get tgt via gpsimd CCE; rest via DVE add

ACOL = 928  # Act engine's zs column slice [0:ACOL); DVE takes the rest

QN = BL * S // 128  # 32 qy rows per partition
QCOLS = QN * V  # 2048

NSTAT = 20
C_AZS = 0  # 4 cols: Act zs accums
C_VZS = 4  # 4 cols: DVE zs accums
C_PTS = 8  # 4 cols: pts-all accums
C_MRK = 12  # 4 cols: pts mark accums
C_KLD = 16
C_BEST = 17

MW = float(np.sqrt(1.0 + ALPHA * P2 / NMARK2))  # 17.2047 best-mark fold

_module = None
last_results = None  # BassKernelResults of the most recent run (for profiling)


def _build_module():
    nc = bacc.Bacc()

    # comb row r=b*S+s : concat(rzs[b,s], pts_perm[b,s], pad) fp8
    comb = nc.dram_tensor("comb", [BL * S, CWP], F8, kind="ExternalInput")
    # tgt row p*16+k : -concat(zs[b,i], gts_perm[b,i], pad), b=p//8, i=16*(p%8)+k
    tgt = nc.dram_tensor("tgt", [128 * BL, CWP], F8, kind="ExternalInput")
    qy = nc.dram_tensor("qy", [BL * S, V], BF16, kind="ExternalInput")
    best = nc.dram_tensor("best", [BL, P2], F32, kind="ExternalInput")
    best_gt = nc.dram_tensor("best_gt", [BL, P2], F32, kind="ExternalInput")
    # idx16[:, 32c:32c+32]: chunk c's dma_gather indices ([16,32] int16
    # blocks, flat index i at [i%16, i//16], replicated over the 8 q7 cores)
    idx16 = nc.dram_tensor("idx16", [128, 32 * NCH], I16, kind="ExternalInput")
    # idxm[p, j] = p*16 + j for j in [0, 4*NMERGE): tgt row merged onto
    # partition p slot j by the CCE ops
    idxm = nc.dram_tensor("idxm", [128, max(1, KC * NMERGE)], I32, kind="ExternalInput")
    out = nc.dram_tensor("out", [128, NSTAT], F32, kind="ExternalOutput")

    with tile.TileContext(nc) as tc:
        with tc.tile_pool(name="cst", bufs=1) as cst:
            nc.gpsimd.load_library(library_config.mlp)

            idx16_t = cst.tile([128, 32 * NCH], I16)
            nc.sync.dma_start(idx16_t[:], idx16[:])
            idxm_t = cst.tile([128, max(1, KC * NMERGE)], I32)
            nc.sync.dma_start(idxm_t[:], idxm[:])

            stats = cst.tile([128, NSTAT], F32)
            nc.vector.memset(stats[:], 0.0)
            ebias = cst.tile([128, 1], F32)
            nc.vector.memset(ebias[:], float(V) * EPS)

            ch = [
                cst.tile([128, CHW], F8, tag=f"ch{c}", name=f"ch{c}")
                for c in range(NCH)
            ]
            # direct tgt tiles for the DVE-subtracted chunks
            tgt_r = tgt[:].rearrange("(p k) c -> p (k c)", k=BL)
            tg = {}
            for c in range(NMERGE, NCH):
                t = cst.tile([128, CHW], F8, tag=f"tg{c}", name=f"tg{c}")
                nc.sync.dma_start(t[:], tgt_r[:, c * CHW : (c + 1) * CHW])
                tg[c] = t
            qy_t = cst.tile([128, QCOLS], BF16)
            nc.sync.dma_start(qy_t[:], qy[:].rearrange("(p n) v -> p (n v)", n=QN))
            bt = cst.tile([BL, P2], F32)
            nc.sync.dma_start(bt[:], best[:])
            bgt = cst.tile([BL, P2], F32)
            nc.sync.dma_start(bgt[:], best_gt[:])

            # ---- gpsimd queue: 4 dma_gathers, then CCE-merge chunks 0-1 ---
            for c in range(NCH):
                nc.gpsimd.dma_gather(
                    ch[c][:].rearrange("p (k w) -> p k w", w=CWP),
                    comb[:],
                    idx16_t[:, 32 * c : 32 * (c + 1)],
                    KC * 128,
                    KC * 128,
                    CWP,
                )
            if NMERGE:
                # the CCE-indirect ucode lives in the default overlay, not
                # mlp: switch back before the merge ops
                nc.gpsimd.load_library(library_config.standard)
            for c in range(NMERGE):
                for k in range(KC):
                    j = c * KC + k
                    nc.gpsimd.indirect_dma_start(
                        out=ch[c][:, k * CWP : (k + 1) * CWP],
                        out_offset=None,
                        in_=tgt[:],
                        in_offset=bass.IndirectOffsetOnAxis(
                            ap=idxm_t[:, j : j + 1], axis=0
                        ),
                        compute_op=OP.add,
                    )

            # ---- DVE: apply tgt to chunks 2-3 (add of negated target) -----
            for c in range(NMERGE, NCH):
                nc.vector.tensor_tensor(
                    out=ch[c][:], in0=ch[c][:], in1=tg[c][:], op=OP.add
                )

            # ---- Act: Ln(qy) then zs column slices [0:ACOL) ---------------
            lg = cst.tile([128, QCOLS], BF16)
            nc.scalar.activation(lg[:], qy_t[:], ACTF.Ln, bias=ebias[:], scale=float(V))
            scr_a = cst.tile([128, KC * ACOL], BF16)
            sa3 = scr_a[:].rearrange("p (k w) -> p k w", w=ACOL)
            act_order = list(range(NMERGE, NCH)) + list(range(NMERGE))
            for c in act_order:
                c3 = ch[c][:].rearrange("p (k w) -> p k w", w=CWP)
                nc.scalar.activation(
                    sa3, c3[:, :, 0:ACOL], ACTF.Square,
                    accum_out=stats[:, C_AZS + c : C_AZS + c + 1],
                )

            # ---- DVE: kld, zs tails, pts, marks, best ---------------------
            scr_k = cst.tile([128, QCOLS], BF16)
            scr_v = cst.tile([128, 2600], BF16)
            DVW = D - ACOL
            sv_z = scr_v[:, : KC * DVW].rearrange("p (k w) -> p k w", w=DVW)
            sv_p = scr_v[:, KC * DVW : KC * DVW + KC * P2].rearrange(
                "p (k w) -> p k w", w=P2
            )
            sv_m = scr_v[:, 2536 : 2536 + KC * NMARK2].rearrange(
                "p (k w) -> p k w", w=NMARK2
            )

            def sq_acc(out_ap, in_ap, acc):
                nc.vector.scalar_tensor_tensor(
                    out=out_ap, in0=in_ap, scalar=0.0, in1=in_ap,
                    op0=OP.subtract, op1=OP.mult, accum_out=acc,
                )

            # kld: sum q * ln(V*q + V*eps) = sum q*(ln q - ln(1/V))
            nc.vector.scalar_tensor_tensor(
                out=scr_k[:], in0=lg[:], scalar=0.0, in1=qy_t[:],
                op0=OP.subtract, op1=OP.mult,
                accum_out=stats[:, C_KLD : C_KLD + 1],
            )
            for c in act_order:
                c3 = ch[c][:].rearrange("p (k w) -> p k w", w=CWP)
                sq_acc(sv_z, c3[:, :, ACOL:D], stats[:, C_VZS + c : C_VZS + c + 1])
                sq_acc(sv_p, c3[:, :, D:CW], stats[:, C_PTS + c : C_PTS + c + 1])
                sq_acc(
                    sv_m, c3[:, :, D : D + NMARK2],
                    stats[:, C_MRK + c : C_MRK + c + 1],
                )

            # best (mark weights folded into column scales on host)
            nc.vector.tensor_sub(bt[:], bt[:], bgt[:])
            sq_acc(bgt[:], bt[:], stats[:BL, C_BEST : C_BEST + 1])

            nc.sync.dma_start(out[:], stats[:])

    nc.compile()
    return nc


def kernel(
    zs, rzs, pts, best, qy, gts, best_gt, mapping, vector_dims, **trace_kwargs
):
    global _module, last_results
    vd = int(np.asarray(vector_dims))
    assert vd == V, f"kernel compiled for vector_dims={V}, got {vd}"

    if _module is None:
        _module = _build_module()

    F8N = ml_dtypes.float8_e4m3
    BF = ml_dtypes.bfloat16
    mapping = np.asarray(mapping).astype(np.int32)
    qy = np.asarray(qy, dtype=np.float32).astype(BF)

    # point-column permutation: the 8 mark columns first
    rest = [i for i in range(P) if i not in MARK]
    perm = np.array(list(MARK) + rest)

    pts_p = np.asarray(pts, dtype=np.float32)[:, :, perm, :].reshape(B, S, P2)
    gts_p = np.asarray(gts, dtype=np.float32)[:, :, perm, :].reshape(B, SG, P2)
    zs = np.asarray(zs, dtype=np.float32)
    rzs = np.asarray(rzs, dtype=np.float32)

    comb = np.zeros((B, S, CWP), dtype=F8N)
    comb[:, :, :D] = rzs
    comb[:, :, D:CW] = pts_p
    tgt = np.zeros((B, SG, CWP), dtype=F8N)
    tgt[:, :, :D] = -zs[:, :SG]
    tgt[:, :, D:CW] = -gts_p

    # best: fold the 10x landmark mse into column scales (f32, no overflow)
    wcol = np.ones(P2, np.float32)
    wcol[2 * np.array(MARK)] = MW
    wcol[2 * np.array(MARK) + 1] = MW
    best2 = np.asarray(best, dtype=np.float32).reshape(B, P2) * wcol
    bgt2 = np.asarray(best_gt, dtype=np.float32).reshape(B, P2) * wcol

    pp = np.arange(128)
    b = pp // 8
    pos = 16 * (pp % 8)[:, None] + np.arange(BL)[None, :]
    idxm = (
        pp[:, None] * BL + np.arange(max(1, KC * NMERGE))[None, :]
    ).astype(np.int32)
    in_maps = []
    for c in range(NCORES):
        sl = slice(c * BL, (c + 1) * BL)
        mp = mapping[sl]  # [BL, SG]
        idx2 = (b[:, None] * S + mp[b[:, None], pos]).astype(np.int16)  # [128, 16]
        # chunk cc's dma_gather wants flat[j*128+p] = idx2[p, 4cc+j]
        idx16 = np.empty((128, 32 * NCH), np.int16)
        for cc in range(NCH):
            fl = idx2[:, 4 * cc : 4 * cc + 4].T.reshape(-1)  # [512] j-major
            idx16[:, 32 * cc : 32 * cc + 32] = np.tile(fl.reshape(32, 16).T, (8, 1))
        # tgt rows: [16b, 8g, 16k, CWP] -> row (b*8+g)*16+k = partition
        # p=b*8+g slot k, i.e. sample i = 16*g+k
        tgt_c = tgt[sl].reshape(BL, 8, BL, CWP)
        in_maps.append(
            {
                "comb": comb[sl].reshape(BL * S, CWP),
                "tgt": np.ascontiguousarray(tgt_c).reshape(128 * BL, CWP),
                "qy": qy[sl].reshape(BL * S, V),
                "best": np.ascontiguousarray(best2[sl]),
                "best_gt": np.ascontiguousarray(bgt2[sl]),
                "idx16": np.ascontiguousarray(idx16),
                "idxm": np.ascontiguousarray(idxm),
            }
        )

    last_results = run_bass_kernel_spmd(
        _module, in_maps, list(range(NCORES)), **trace_kwargs
    )
    tot = np.zeros(NSTAT, np.float64)
    for r in last_results.results:
        tot += np.asarray(r["out"], dtype=np.float64).reshape(128, NSTAT).sum(axis=0)

    a_zs = tot[C_AZS : C_AZS + NCH].sum() + tot[C_VZS : C_VZS + NCH].sum()
    ae_loss = a_zs / (B * SG * D)
    bias_loss = tot[C_PTS : C_PTS + NCH].sum() / (B * SG * P2) + ALPHA * tot[
        C_MRK : C_MRK + NCH
    ].sum() / (B * SG * NMARK2)
    kld_loss = tot[C_KLD] / (B * S)
    best_mse = tot[C_BEST] / (B * P2)

    return np.array(kld_loss + ae_loss + best_mse + bias_loss, dtype=np.float32)


# revision 14
# speedup vs baseline: 1.3348x; 1.3348x over previous
"""CQVAE loss kernel for Trainium2, data-parallel over batch on 8 NeuronCores.

loss = kld(qy) + mse(gather(rzs), zs[:, :Sg]) + bias(best, best_gt)
       + bias(gather(pts), gts)
where bias(p, g) = mse(p, g) + 10 * mse(p[..., MARK, :], g[..., MARK, :]).

Design (per core, 16 of 128 batches):
- Host packs comb rows [rzs | pts(mark-cols-first) | pad->1280B] and a
  NEGATED target [zs | gts(mark-cols-first) | pad], both fp8e4m3
  (tolerance 2e-2 >> fp8 noise; rel err lands ~1e-3), halving HBM
  traffic vs bf16.
- The mapping-gather runs as 4 dma_gather ops (512 rows each, ~1.5us of
  SWDGE emission apiece) instead of 16 indirect ops (~1.8us per 128
  rows) - the decisive change, since gather emission on the gpsimd
  queue is the kernel's critical path.
- The negated target is applied two ways, chosen per chunk to balance
  engines: chunks 0-1 via indirect-CCE ops (descriptor = one 1260B tgt
  row, the only DMA-accumulate shape the ucode supports) on the
  otherwise-idle gpsimd queue; chunks 2-3 via direct loads + one DVE
  add per chunk (tgt is pre-negated so add == subtract).
- d**2 + accumulate splits between Act (zs columns [0:ACOL), Square
  with accum) and DVE (zs tail, pts, marks, kld, best); landmark
  weights use separate accumulators (marks permuted to the front of the
  pts block) or host-folded column scales (best), so no per-landmark
  reduction ops.
"""

import sys

import numpy as np

try:
    import concourse  # noqa: F401
except ImportError:  # pragma: no cover
    sys.path.insert(0, "/opt/trn_rl_repo")

import ml_dtypes

import concourse.bass as bass  # noqa: F401
import concourse.mybir as mybir
import concourse.tile as tile
from concourse import bacc, library_config
from concourse.bass_utils import run_bass_kernel_spmd

F32 = mybir.dt.float32
BF16 = mybir.dt.bfloat16
F8 = mybir.dt.float8e4
I32 = mybir.dt.int32
I16 = mybir.dt.int16
AX = mybir.AxisListType
OP = mybir.AluOpType
ACTF = mybir.ActivationFunctionType

NCORES = 8
B, S, SG, D, P, V = 128, 256, 128, 1024, 118, 64
BL = B // NCORES  # batches per core
P2 = 2 * P  # 236 point floats per row
CW = D + P2  # 1260 live row floats
CWP = 1280  # padded row width (dma_gather wants %256 bytes)
MARK = (0, 29, 88, 117)
NMARK2 = 2 * len(MARK)  # 8 mark columns
EPS = 1e-20
ALPHA = 10.0

NCH = 4  # chunks (4 slots each)
KC = BL // NCH
CHW = KC * CWP  # 5120 cols per chunk tile
NMERGE = 0  # chunks 0..NMERGE-1 get tgt via gpsimd CCE; rest via DVE add

ACOL = 928  # Act engine's zs column slice [0:ACOL); DVE takes the rest

QN = BL * S // 128  # 32 qy rows per partition
QCOLS = QN * V  # 2048

NSTAT = 20
C_AZS = 0  # 4 cols: Act zs accums
C_VZS = 4  # 4 cols: DVE zs accums
C_PTS = 8  # 4 cols: pts-all accums
C_MRK = 12  # 4 cols: pts mark accums
C_KLD = 16
C_BEST = 17

MW = float(np.sqrt(1.0 + ALPHA * P2 / NMARK2))  # 17.2047 best-mark fold

_module = None
last_results = None  # BassKernelResults of the most recent run (for profiling)


def _build_module():
    nc = bacc.Bacc()

    # comb row r=b*S+s : concat(rzs[b,s], pts_perm[b,s], pad) fp8
    comb = nc.dram_tensor("comb", [BL * S, CWP], F8, kind="ExternalInput")
    # tgt row p*16+k : -concat(zs[b,i], gts_perm[b,i], pad), b=p//8, i=16*(p%8)+k
    tgt = nc.dram_tensor("tgt", [128 * BL, CWP], F8, kind="ExternalInput")
    qy = nc.dram_tensor("qy", [BL * S, V], BF16, kind="ExternalInput")
    best = nc.dram_tensor("best", [BL, P2], F32, kind="ExternalInput")
    best_gt = nc.dram_tensor("best_gt", [BL, P2], F32, kind="ExternalInput")
    # idx16[:, 32c:32c+32]: chunk c's dma_gather indices ([16,32] int16
    # blocks, flat index i at [i%16, i//16], replicated over the 8 q7 cores)
    idx16 = nc.dram_tensor("idx16", [128, 32 * NCH], I16, kind="ExternalInput")
    # idxm[p, j] = p*16 + j for j in [0, 4*NMERGE): tgt row merged onto
    # partition p slot j by the CCE ops
    idxm = nc.dram_tensor("idxm", [128, max(1, KC * NMERGE)], I32, kind="ExternalInput")
    out = nc.dram_tensor("out", [128, NSTAT], F32, kind="ExternalOutput")

    with tile.TileContext(nc) as tc:
        with tc.tile_pool(name="cst", bufs=1) as cst:
            nc.gpsimd.load_library(library_config.mlp)

            idx16_t = cst.tile([128, 32 * NCH], I16)
            nc.sync.dma_start(idx16_t[:], idx16[:])
            idxm_t = cst.tile([128, max(1, KC * NMERGE)], I32)
            nc.sync.dma_start(idxm_t[:], idxm[:])

            stats = cst.tile([128, NSTAT], F32)
            nc.vector.memset(stats[:], 0.0)
            ebias = cst.tile([128, 1], F32)
            nc.vector.memset(ebias[:], float(V) * EPS)

            ch = [
                cst.tile([128, CHW], F8, tag=f"ch{c}", name=f"ch{c}")
                for c in range(NCH)
            ]
            # direct tgt tiles for the DVE-subtracted chunks
            tgt_r = tgt[:].rearrange("(p k) c -> p (k c)", k=BL)
            tg = {}
            for c in range(NMERGE, NCH):
                t = cst.tile([128, CHW], F8, tag=f"tg{c}", name=f"tg{c}")
                nc.sync.dma_start(t[:], tgt_r[:, c * CHW : (c + 1) * CHW])
                tg[c] = t
            qy_t = cst.tile([128, QCOLS], BF16)
            nc.sync.dma_start(qy_t[:], qy[:].rearrange("(p n) v -> p (n v)", n=QN))
            bt = cst.tile([BL, P2], F32)
            nc.sync.dma_start(bt[:], best[:])
            bgt = cst.tile([BL, P2], F32)
            nc.sync.dma_start(bgt[:], best_gt[:])

            # ---- gpsimd queue: 4 dma_gathers, then CCE-merge chunks 0-1 ---
            for c in range(NCH):
                nc.gpsimd.dma_gather(
                    ch[c][:].rearrange("p (k w) -> p k w", w=CWP),
                    comb[:],
                    idx16_t[:, 32 * c : 32 * (c + 1)],
                    KC * 128,
                    KC * 128,
                    CWP,
                )
            if NMERGE:
                # the CCE-indirect ucode lives in the default overlay, not
                # mlp: switch back before the merge ops
                nc.gpsimd.load_library(library_config.standard)
            for c in range(NMERGE):
                for k in range(KC):
                    j = c * KC + k
                    nc.gpsimd.indirect_dma_start(
                        out=ch[c][:, k * CWP : (k + 1) * CWP],
                        out_offset=None,
                        in_=tgt[:],
                        in_offset=bass.IndirectOffsetOnAxis(
                            ap=idxm_t[:, j : j + 1], axis=0
                        ),
                        compute_op=OP.add,
                    )

            # ---- DVE: apply tgt to chunks 2-3 (add of negated target) -----
            for c in range(NMERGE, NCH):
                nc.vector.tensor_tensor(
                    out=ch[c][:], in0=ch[c][:], in1=tg[c][:], op=OP.add
                )

            # ---- Act: Ln(qy) then zs column slices [0:ACOL) ---------------
            lg = cst.tile([128, QCOLS], BF16)
            nc.scalar.activation(lg[:], qy_t[:], ACTF.Ln, bias=ebias[:], scale=float(V))
            scr_a = cst.tile([128, KC * ACOL], BF16)
            sa3 = scr_a[:].rearrange("p (k w) -> p k w", w=ACOL)
            act_order = list(range(NMERGE, NCH)) + list(range(NMERGE))
            for c in act_order:
                c3 = ch[c][:].rearrange("p (k w) -> p k w", w=CWP)
                nc.scalar.activation(
                    sa3, c3[:, :, 0:ACOL], ACTF.Square,
                    accum_out=stats[:, C_AZS + c : C_AZS + c + 1],
                )

            # ---- DVE: kld, zs tails, pts, marks, best ---------------------
            scr_k = cst.tile([128, QCOLS], BF16)
            scr_v = cst.tile([128, 2600], BF16)
            DVW = D - ACOL
            sv_z = scr_v[:, : KC * DVW].rearrange("p (k w) -> p k w", w=DVW)
            sv_p = scr_v[:, KC * DVW : KC * DVW + KC * P2].rearrange(
                "p (k w) -> p k w", w=P2
            )
            sv_m = scr_v[:, 2536 : 2536 + KC * NMARK2].rearrange(
                "p (k w) -> p k w", w=NMARK2
            )

            def sq_acc(out_ap, in_ap, acc):
                nc.vector.scalar_tensor_tensor(
                    out=out_ap, in0=in_ap, scalar=0.0, in1=in_ap,
                    op0=OP.subtract, op1=OP.mult, accum_out=acc,
                )

            # kld: sum q * ln(V*q + V*eps) = sum q*(ln q - ln(1/V))
            nc.vector.scalar_tensor_tensor(
                out=scr_k[:], in0=lg[:], scalar=0.0, in1=qy_t[:],
                op0=OP.subtract, op1=OP.mult,
                accum_out=stats[:, C_KLD : C_KLD + 1],
            )
            for c in act_order:
                c3 = ch[c][:].rearrange("p (k w) -> p k w", w=CWP)
                sq_acc(sv_z, c3[:, :, ACOL:D], stats[:, C_VZS + c : C_VZS + c + 1])
                sq_acc(sv_p, c3[:, :, D:CW], stats[:, C_PTS + c : C_PTS + c + 1])
                sq_acc(
                    sv_m, c3[:, :, D : D + NMARK2],
                    stats[:, C_MRK + c : C_MRK + c + 1],
                )

            # best (mark weights folded into column scales on host)
            nc.vector.tensor_sub(bt[:], bt[:], bgt[:])
            sq_acc(bgt[:], bt[:], stats[:BL, C_BEST : C_BEST + 1])

            nc.sync.dma_start(out[:], stats[:])

    nc.compile()
    return nc


def kernel(
    zs, rzs, pts, best, qy, gts, best_gt, mapping, vector_dims, **trace_kwargs
):
    global _module, last_results
    vd = int(np.asarray(vector_dims))
    assert vd == V, f"kernel compiled for vector_dims={V}, got {vd}"

    if _module is None:
        _module = _build_module()

    F8N = ml_dtypes.float8_e4m3
    BF = ml_dtypes.bfloat16
    mapping = np.asarray(mapping).astype(np.int32)
    qy = np.asarray(qy, dtype=np.float32).astype(BF)

    # point-column permutation: the 8 mark columns first
    rest = [i for i in range(P) if i not in MARK]
    perm = np.array(list(MARK) + rest)

    pts_p = np.asarray(pts, dtype=np.float32)[:, :, perm, :].reshape(B, S, P2)
    gts_p = np.asarray(gts, dtype=np.float32)[:, :, perm, :].reshape(B, SG, P2)
    zs = np.asarray(zs, dtype=np.float32)
    rzs = np.asarray(rzs, dtype=np.float32)

    comb = np.zeros((B, S, CWP), dtype=F8N)
    comb[:, :, :D] = rzs
    comb[:, :, D:CW] = pts_p
    tgt = np.zeros((B, SG, CWP), dtype=F8N)
    tgt[:, :, :D] = -zs[:, :SG]
    tgt[:, :, D:CW] = -gts_p

    # best: fold the 10x landmark mse into column scales (f32, no overflow)
    wcol = np.ones(P2, np.float32)
    wcol[2 * np.array(MARK)] = MW
    wcol[2 * np.array(MARK) + 1] = MW
    best2 = np.asarray(best, dtype=np.float32).reshape(B, P2) * wcol
    bgt2 = np.asarray(best_gt, dtype=np.float32).reshape(B, P2) * wcol

    pp = np.arange(128)
    b = pp // 8
    pos = 16 * (pp % 8)[:, None] + np.arange(BL)[None, :]
    idxm = (
        pp[:, None] * BL + np.arange(max(1, KC * NMERGE))[None, :]
    ).astype(np.int32)
    in_maps = []
    for c in range(NCORES):
        sl = slice(c * BL, (c + 1) * BL)
        mp = mapping[sl]  # [BL, SG]
        idx2 = (b[:, None] * S + mp[b[:, None], pos]).astype(np.int16)  # [128, 16]
        # chunk cc's dma_gather wants flat[j*128+p] = idx2[p, 4cc+j]
        idx16 = np.empty((128, 32 * NCH), np.int16)
        for cc in range(NCH):
            fl = idx2[:, 4 * cc : 4 * cc + 4].T.reshape(-1)  # [512] j-major
            idx16[:, 32 * cc : 32 * cc + 32] = np.tile(fl.reshape(32, 16).T, (8, 1))
        # tgt rows: [16b, 8g, 16k, CWP] -> row (b*8+g)*16+k = partition
        # p=b*8+g slot k, i.e. sample i = 16*g+k
        tgt_c = tgt[sl].reshape(BL, 8, BL, CWP)
        in_maps.append(
            {
                "comb": comb[sl].reshape(BL * S, CWP),
                "tgt": np.ascontiguousarray(tgt_c).reshape(128 * BL, CWP),
                "qy": qy[sl].reshape(BL * S, V),
                "best": np.ascontiguousarray(best2[sl]),
                "best_gt": np.ascontiguousarray(bgt2[sl]),
                "idx16": np.ascontiguousarray(idx16),
                "idxm": np.ascontiguousarray(idxm),
            }
        )

    last_results = run_bass_kernel_spmd(
        _module, in_maps, list(range(NCORES)), **trace_kwargs
    )
    tot = np.zeros(NSTAT, np.float64)
    for r in last_results.results:
        tot += np.asarray(r["out"], dtype=np.float64).reshape(128, NSTAT).sum(axis=0)

    a_zs = tot[C_AZS : C_AZS + NCH].sum() + tot[C_VZS : C_VZS + NCH].sum()
    ae_loss = a_zs / (B * SG * D)
    bias_loss = tot[C_PTS : C_PTS + NCH].sum() / (B * SG * P2) + ALPHA * tot[
        C_MRK : C_MRK + NCH
    ].sum() / (B * SG * NMARK2)
    kld_loss = tot[C_KLD] / (B * S)
    best_mse = tot[C_BEST] / (B * P2)

    return np.array(kld_loss + ae_loss + best_mse + bias_loss, dtype=np.float32)


# revision 15
# speedup vs baseline: 1.6399x; 1.2286x over previous
"""CQVAE loss kernel for Trainium2, data-parallel over batch on 8 NeuronCores.

loss = kld(qy) + mse(gather(rzs), zs[:, :Sg]) + bias(best, best_gt)
       + bias(gather(pts), gts)
where bias(p, g) = mse(p, g) + 10 * mse(p[..., MARK, :], g[..., MARK, :]).

Each core handles 16 of the 128 batches.  The mapping-gathers run as
dma_gather ops (hundreds of rows per op, ~9ns/row of Q7 emission)
interleaved so gather bytes, zs bytes and compute pipeline smoothly.
pts/gts rows are zero-padded to 256 floats on the host so gathered rows
are 1KB-aligned and pad columns contribute nothing to the sums.  zs/gts
are laid out so every partition reads one contiguous 64/16KB run.  Each
core ships a [128, 32] per-partition stats tile; the host folds
partitions and cores.
"""

import sys

import numpy as np

try:
    import concourse  # noqa: F401
except ImportError:  # pragma: no cover
    sys.path.insert(0, "/opt/trn_rl_repo")

import ml_dtypes

import concourse.bass as bass  # noqa: F401
import concourse.mybir as mybir
import concourse.tile as tile
from concourse import bacc
from concourse.bass_utils import run_bass_kernel_spmd

F32 = mybir.dt.float32
BF16 = mybir.dt.bfloat16
I32 = mybir.dt.int32
AX = mybir.AxisListType
OP = mybir.AluOpType
ACTF = mybir.ActivationFunctionType

NCORES = 8
B, S, SG, D, P, V = 128, 256, 128, 1024, 118, 64
BL = B // NCORES  # batches per core
P2 = 2 * P  # 236 true floats per point-row
PC = 256  # padded point-row width
MARK = (0, 29, 88, 117)
EPS = 1e-20
ALPHA = 10.0

NSTAT = 36
# stats columns
C_KLD = 33
C_BEST, C_BESTM = 10, 11
C_AE = 0  # 10 cols: ae pieces
C_BIAS = 12  # 4 cols: bias sq totals per pts quarter
C_MARK = 16  # 16 cols: 4 marks x 4 quarters

CW = D + PC  # 1280 combined row width

# rzs pieces by (start_slot, n_slots): coarse early, 1-slot at the end
AE_PIECES = [(0, 2), (2, 2), (4, 2), (6, 2), (8, 2), (10, 2),
             (12, 1), (13, 1), (14, 1), (15, 1)]
NAE = len(AE_PIECES)
NPT = 4  # pts gather ops / gts quarters (4 batches each)
KP = BL // NPT  # 4 batch-slots per pts quarter

_module = None
last_results = None  # BassKernelResults of the most recent run (for profiling)


def _build_module():
    nc = bacc.Bacc()

    zs = nc.dram_tensor("zs", [BL * SG, D], BF16, kind="ExternalInput")
    # comb row r = concat(rzs[r], pts_padded[r]) — one gather fetches both
    comb = nc.dram_tensor("comb", [BL * S, CW], BF16, kind="ExternalInput")
    gts = nc.dram_tensor("gts", [BL * SG, PC], BF16, kind="ExternalInput")
    qy = nc.dram_tensor("qy", [BL * S, V], BF16, kind="ExternalInput")
    best = nc.dram_tensor("best", [BL, P2], F32, kind="ExternalInput")
    best_gt = nc.dram_tensor("best_gt", [BL, P2], F32, kind="ExternalInput")
    # idx[p, k] = (p//8)*S + mapping[p//8, 16*(p%8) + k] — the flat source
    # row for slot k of partition p, shared by the rzs and pts gathers
    idx2 = nc.dram_tensor("idx2", [128, BL], I32, kind="ExternalInput")
    out = nc.dram_tensor("out", [128, NSTAT], F32, kind="ExternalOutput")

    QCOLS = BL * S * V // 128  # 2048
    QN = BL * S // 128  # 32 qy rows per partition

    with tile.TileContext(nc) as tc:
        with tc.tile_pool(name="cst", bufs=1) as cst:
            idx_t = cst.tile([128, BL], I32)
            nc.sync.dma_start(idx_t[:], idx2[:])

            stats = cst.tile([128, NSTAT], F32)
            nc.vector.memset(stats[:], 0.0)

            # ---- gathers: one combined-row op per slot (SWDGE queue) ------
            cb = cst.tile([128, BL * CW], BF16)
            for k in range(BL):
                nc.gpsimd.indirect_dma_start(
                    out=cb[:, k * CW : (k + 1) * CW],
                    out_offset=None,
                    in_=comb[:],
                    in_offset=bass.IndirectOffsetOnAxis(
                        ap=idx_t[:, k : k + 1], axis=0
                    ),
                )
            cb3 = cb[:].rearrange("p (k c) -> p k c", c=CW)

            # ---- direct loads --------------------------------------------
            # scalar HWDGE queue: qy, best, gts quarters (3.2 MB)
            qy_t = cst.tile([128, QCOLS], BF16)
            nc.scalar.dma_start(
                qy_t[:], qy[:].rearrange("(p n) v -> p (n v)", n=QN)
            )
            bt = cst.tile([BL, P2], F32)
            nc.scalar.dma_start(bt[:], best[:])
            bgt = cst.tile([BL, P2], F32)
            nc.scalar.dma_start(bgt[:], best_gt[:])
            # partition p holds gts rows 16p..16p+15 (contiguous 16KB)
            gts_r = gts[:].rearrange("(p k) c -> p (k c)", k=BL)
            gt_h = []
            for h in range(NPT):
                g = cst.tile([128, KP * PC], BF16, tag=f"gt{h}", name=f"gt{h}")
                nc.scalar.dma_start(g[:], gts_r[:, h * KP * PC : (h + 1) * KP * PC])
                gt_h.append(g)

            # sync HWDGE queue: zs pieces (8.4 MB)
            # partition p holds zs rows 16p..16p+15 (contiguous 64KB)
            zs_r = zs[:].rearrange("(p k) d -> p (k d)", k=BL)
            zs_t = []
            for j, (s0, ns) in enumerate(AE_PIECES):
                z = cst.tile([128, ns * D], BF16, tag=f"zs{j}", name=f"zs{j}")
                nc.sync.dma_start(z[:], zs_r[:, s0 * D : (s0 + ns) * D])
                zs_t.append(z)

            # ---- compute --------------------------------------------------
            # BEST (tiny, lands early on the scalar queue)
            nc.vector.tensor_sub(bt[:], bt[:], bgt[:])
            nc.vector.tensor_mul(bt[:], bt[:], bt[:])
            nc.vector.reduce_sum(out=stats[:BL, C_BEST : C_BEST + 1], in_=bt[:], axis=AX.X)
            bm4 = cst.tile([BL, 4], F32)
            for j, m in enumerate(MARK):
                nc.vector.reduce_sum(
                    out=bm4[:, j : j + 1], in_=bt[:, 2 * m : 2 * m + 2], axis=AX.X
                )
            nc.vector.reduce_sum(out=stats[:BL, C_BESTM : C_BESTM + 1], in_=bm4[:], axis=AX.X)

            # KLD: sum q * (log(q + eps) - log(1/V)) via log(V*q + V*eps)
            lg = cst.tile([128, QCOLS], F32)
            ebias = cst.tile([128, 1], F32)
            nc.vector.memset(ebias[:], float(V) * EPS)
            nc.scalar.activation(lg[:], qy_t[:], ACTF.Ln, bias=ebias[:], scale=float(V))
            nc.vector.scalar_tensor_tensor(
                out=lg[:],
                in0=lg[:],
                scalar=0.0,
                in1=qy_t[:],
                op0=OP.subtract,
                op1=OP.mult,
                accum_out=stats[:, C_KLD : C_KLD + 1],
            )

            def ae_piece(j, on_dve=False):
                s0, ns = AE_PIECES[j]
                rg = cb3[:, s0 : s0 + ns, 0:D]
                z3 = zs_t[j][:].rearrange("p (k d) -> p k d", d=D)
                nc.vector.tensor_sub(rg, rg, z3)
                acc = stats[:, C_AE + j : C_AE + j + 1]
                if on_dve:
                    # square-and-accumulate on DVE; the consumed zs tile is
                    # the scratch destination (no operand aliasing)
                    nc.vector.scalar_tensor_tensor(
                        out=z3, in0=rg, scalar=0.0, in1=rg,
                        op0=OP.subtract, op1=OP.mult, accum_out=acc,
                    )
                else:
                    nc.scalar.activation(rg, rg, ACTF.Square, accum_out=acc)

            def bias_quarter(h, on_dve=False):
                pg = cb3[:, h * KP : (h + 1) * KP, D : D + PC]
                g3 = gt_h[h][:].rearrange("p (k c) -> p k c", c=PC)
                nc.vector.tensor_sub(pg, pg, g3)
                acc = stats[:, C_BIAS + h : C_BIAS + h + 1]
                if on_dve:
                    # squares land in the consumed gts tile; marks read there
                    nc.vector.scalar_tensor_tensor(
                        out=g3, in0=pg, scalar=0.0, in1=pg,
                        op0=OP.subtract, op1=OP.mult, accum_out=acc,
                    )
                    sq = g3
                else:
                    nc.scalar.activation(pg, pg, ACTF.Square, accum_out=acc)
                    sq = pg
                cm = C_MARK + 4 * h
                for j, m in enumerate(MARK):
                    nc.vector.reduce_sum(
                        out=stats[:, cm + j : cm + j + 1],
                        in_=sq[:, :, 2 * m : 2 * m + 2],
                        axis=AX.XY,
                    )

            # compute in data-arrival order
            ae_piece(0)
            ae_piece(1)
            bias_quarter(0)
            ae_piece(2)
            bias_quarter(1)
            ae_piece(3)
            bias_quarter(2)
            ae_piece(4)
            bias_quarter(3, on_dve=True)
            ae_piece(5)
            ae_piece(6, on_dve=True)
            ae_piece(7)
            ae_piece(8, on_dve=True)
            ae_piece(9)

            nc.sync.dma_start(out[:], stats[:])

    nc.compile()
    return nc


def kernel(
    zs, rzs, pts, best, qy, gts, best_gt, mapping, vector_dims, **trace_kwargs
):
    global _module, last_results
    vd = int(np.asarray(vector_dims))
    assert vd == V, f"kernel compiled for vector_dims={V}, got {vd}"

    if _module is None:
        _module = _build_module()

    BF = ml_dtypes.bfloat16
    zs = np.asarray(zs, dtype=np.float32)
    qy = np.asarray(qy, dtype=np.float32).astype(BF)
    mapping = np.asarray(mapping).astype(np.int32)
    best2 = np.ascontiguousarray(np.asarray(best, dtype=np.float32).reshape(B, P2))
    bgt2 = np.ascontiguousarray(np.asarray(best_gt, dtype=np.float32).reshape(B, P2))

    # combined gather rows: [rzs | pts zero-padded to PC], bf16
    comb = np.zeros((B, S, CW), dtype=BF)
    comb[:, :, :D] = np.asarray(rzs, dtype=np.float32).reshape(B, S, D).astype(BF)
    comb[:, :, D : D + P2] = (
        np.asarray(pts, dtype=np.float32).reshape(B, S, P2).astype(BF)
    )
    gts_p = np.zeros((B, SG, PC), dtype=BF)
    gts_p[:, :, :P2] = np.asarray(gts, dtype=np.float32).reshape(B, SG, P2).astype(BF)

    pp = np.arange(128)
    in_maps = []
    for c in range(NCORES):
        sl = slice(c * BL, (c + 1) * BL)
        mp = mapping[sl]  # [BL, SG]
        b = pp // 8
        pos = 16 * (pp % 8)[:, None] + np.arange(BL)[None, :]
        idx2 = (b[:, None] * S + mp[b[:, None], pos]).astype(np.int32)
        # zs rows reordered so partition p holds rows 16p..16p+15:
        # row 16p+k = zs[b, 16q+k] -> natural order already (b-major, i-minor)
        in_maps.append(
            {
                "zs": np.ascontiguousarray(zs[sl, :SG].reshape(BL * SG, D).astype(BF)),
                "comb": comb[sl].reshape(BL * S, CW),
                "gts": gts_p[sl].reshape(BL * SG, PC),
                "qy": qy[sl].reshape(BL * S, V),
                "best": np.ascontiguousarray(best2[sl]),
                "best_gt": np.ascontiguousarray(bgt2[sl]),
                "idx2": np.ascontiguousarray(idx2),
            }
        )

    last_results = run_bass_kernel_spmd(
        _module, in_maps, list(range(NCORES)), **trace_kwargs
    )
    parts = np.stack(
        [
            np.asarray(r["out"], dtype=np.float64).reshape(128, NSTAT).sum(axis=0)
            for r in last_results.results
        ]
    )
    tot = parts.sum(axis=0)

    ae_loss = tot[C_AE : C_AE + NAE].sum() / (B * SG * D)
    bias_sq = tot[C_BIAS : C_BIAS + NPT].sum()
    mark_sq = tot[C_MARK : C_MARK + 4 * NPT].sum()
    bias_loss = bias_sq / (B * SG * P2) + ALPHA * mark_sq / (B * SG * 2 * len(MARK))
    kld_loss = tot[C_KLD] / (B * S)
    best_mse = tot[C_BEST] / (B * P2) + ALPHA * tot[C_BESTM] / (B * 2 * len(MARK))

    return np.array(kld_loss + ae_loss + best_mse + bias_loss, dtype=np.float32)

